# revision 6
# baseline (speedup 1.0000x reference)
"""GCN link-prediction kernel for Trainium2 (8 NeuronCores).

Self-contained. N=100000 nodes, IN_CH=128, HID=16, E=3200000 edges.

Device strategy (transposed-table, ap_gather sweeps):
- Nodes range-sharded over 8 cores; edges partitioned by target owner.
- Tables t = (x~ @ W) stored feature-on-partition, replicated per
  16-partition GPSIMD group; per-edge gathers via InstAPGather over 4
  node windows (int16 index range), padded per (round-of-8-nodes,
  window) rectangles so DVE segment reduces are rectangular.
- dinv folded into gather sources (host folds into x, device folds into
  the layer-2 transform input), so aggregation is an unweighted sum.
- Scoring reuses the same slot streams on the final H table; per-slot
  dot products via DVE mul + PE transpose + DVE reduce; sigmoid on ACT.
- 3 AllGathers (one per table) via gpsimd collective_compute.
"""
import sys
import os

sys.path.insert(0, "/opt/trn_rl_repo")

import numpy as np

N_NODES = 100000
IN_CH = 128
HID = 16
N_EDGES = 3200000
N_CORES = 8
NL = 12500            # real nodes per core
NLP = 12544           # padded (8*1568)
R_ROUNDS = 1568       # rounds of 8 nodes
W_WIN = 4             # table windows
WN = 25088            # nodes per window (2 cores * NLP)
SEG = 3072            # gather slots per group per instruction (mult of 128)

LAST_HW_EXEC_NS = None


# ----------------------------------------------------------------- reference
def _reference_numpy(x, edge_index, W1, b1, W2, b2):
    row = edge_index[0].astype(np.int64)
    col = edge_index[1].astype(np.int64)
    n = x.shape[0]
    deg = np.bincount(col, minlength=n).astype(np.float32) + 1.0
    dinv = 1.0 / np.sqrt(deg)

    def layer(h, W, b):
        hw = h @ W
        g = hw * dinv[:, None]
        agg = np.zeros_like(g)
        np.add.at(agg, col, g[row])
        out = dinv[:, None] * (agg + g) + b
        return out

    h = layer(x, W1, b1)
    h = np.maximum(h, 0.0)
    h = layer(h, W2, b2)
    s = (h[row] * h[col]).sum(axis=1)
    return 1.0 / (1.0 + np.exp(-s))


# ----------------------------------------------------------------- host prep
def _prepare(x, edge_index, W1, b1, W2, b2):
    row = edge_index[0].astype(np.int64)
    col = edge_index[1].astype(np.int64)
    deg = np.bincount(col, minlength=N_NODES).astype(np.float32) + 1.0
    dinv = (1.0 / np.sqrt(deg)).astype(np.float32)

    cc = (col // NL).astype(np.int32)          # owner core of each edge
    rc = (row // NL).astype(np.int32)          # source core
    we = (rc // 2).astype(np.int32)            # window of each edge

    # per-(col, window) counts -> per-core node perm (lexsort by window vec)
    cntkey = col * 4 + we
    cnt = np.bincount(cntkey, minlength=N_NODES * 4).reshape(N_NODES, 4)

    permpos = np.empty(N_NODES, dtype=np.int64)   # global node -> perm slot
    perms = []
    for c in range(N_CORES):
        cn = cnt[c * NL:(c + 1) * NL]
        order = np.lexsort((cn[:, 3], cn[:, 2], cn[:, 1], cn[:, 0]))[::-1]
        perms.append(order)                        # perm slot m -> local node
        permpos[c * NL + order] = np.arange(NL)

    m_col = permpos[col]                           # perm slot of target
    g_e = (m_col % 8).astype(np.int32)
    r_e = (m_col // 8).astype(np.int32)

    # K[r, w] = max over (core, group) of counts, >= 1
    k4key = ((cc.astype(np.int64) * R_ROUNDS + r_e) * 8 + g_e) * 4 + we
    cnt4 = np.bincount(k4key, minlength=N_CORES * R_ROUNDS * 8 * 4)
    cnt4 = cnt4.reshape(N_CORES, R_ROUNDS, 8, 4)
    K = cnt4.max(axis=(0, 2))                      # [R, 4]
    K = np.maximum(K, 1).astype(np.int64)
    # block-max over 4-round blocks: fewer, longer K-runs (fewer DVE ops)
    K = np.repeat(K.reshape(R_ROUNDS // 4, 4, W_WIN).max(axis=1), 4, axis=0)

    # segment structure per window: rounds packed into segments <= SEG,
    # each segment padded to a multiple of 128 slots.
    seg_struct = []      # per w: list of dict(runs=[(r0,R,K,coff)], len, base)
    off_round = np.zeros((R_ROUNDS, W_WIN), dtype=np.int64)  # pos in stream
    L_w = []
    for w in range(W_WIN):
        segs = []
        cur_runs = []
        cur_len = 0
        seg_base = 0
        r = 0
        run_r0, run_k, run_cnt = 0, int(K[0, w]), 0

        def close_run():
            nonlocal run_cnt, cur_runs
            if run_cnt > 0:
                cur_runs.append((run_r0, run_cnt, run_k,
                                 cur_len - run_cnt * run_k))
            run_cnt = 0

        def close_seg():
            nonlocal cur_len, cur_runs, seg_base, segs
            if cur_len == 0:
                return
            plen = -cur_len % 128
            segs.append({"runs": cur_runs, "len": cur_len + plen,
                         "base": seg_base})
            seg_base += cur_len + plen
            cur_runs = []
            cur_len = 0

        while r < R_ROUNDS:
            k = int(K[r, w])
            if cur_len + k > SEG:
                close_run()
                close_seg()
                run_r0, run_k, run_cnt = r, k, 0
            if run_cnt == 0 or k != run_k:
                close_run()
                run_r0, run_k = r, k
            off_round[r, w] = seg_base + cur_len
            cur_len += k
            run_cnt += 1
            r += 1
        close_run()
        close_seg()
        seg_struct.append(segs)
        L_w.append(seg_base)

    # scoring column layout: in sweep order (w, seg): ncols = len/128*8
    colbase = {}
    sc_total = 0
    for w in range(W_WIN):
        for si, s in enumerate(seg_struct[w]):
            colbase[(w, si)] = sc_total
            sc_total += (s["len"] // 128) * 8

    # per-edge slot rank k within (core, w, g, r)
    order = np.lexsort((r_e, g_e, we, cc))
    so = order
    key_sorted = (((cc[so].astype(np.int64) * 4 + we[so]) * 8 + g_e[so])
                  * R_ROUNDS + r_e[so])
    newgrp = np.empty(len(so), dtype=bool)
    newgrp[0] = True
    newgrp[1:] = key_sorted[1:] != key_sorted[:-1]
    starts = np.flatnonzero(newgrp)
    gid = np.cumsum(newgrp) - 1
    k_in = np.arange(len(so)) - starts[gid]
    k_e = np.empty(N_EDGES, dtype=np.int64)
    k_e[so] = k_in

    # device stream position (per group): i = off_round[r, w] + k
    i_e = off_round[r_e, we] + k_e

    # gather index value: window-local gid of source row
    gid_row = rc.astype(np.int64) * NLP + permpos[row]
    widx_e = (gid_row - we.astype(np.int64) * WN).astype(np.int16)

    # build idx arrays per core: [128, sum_w L_w/16] int16, pads -> zero row
    Lsum = sum(L_w)
    idx_off_w = np.cumsum([0] + [lw // 16 for lw in L_w])
    idx_maps = []
    for c in range(N_CORES):
        arr = np.full((128, Lsum // 16), 12500, dtype=np.int16)  # pad idx
        idx_maps.append(arr)
    # scatter edge idx values: core cc, group g, stream pos i, window w:
    # element t of group stream at [16*g + (i%16)?? NO: wrapped (s p):
    # stream elem i at partition 16*g + i%16, col idx_off_w[w] + i//16
    part = (g_e.astype(np.int64) * 16 + (i_e % 16))
    colp = idx_off_w[we] + i_e // 16
    for c in range(N_CORES):
        m = cc == c
        idx_maps[c][part[m], colp[m]] = widx_e[m]

    # x~ = x * dinv, transposed, perm-grouped per core:
    # xt[c][:, g*1568 + r] = x~[node at slot 8r+g of core c]; fakes -> 0
    xs = (x.astype(np.float32) * dinv[:, None])
    xts = []
    for c in range(N_CORES):
        nodes = c * NL + perms[c]                  # perm slot m -> global
        xt = np.zeros((IN_CH, NLP), dtype=np.float32)
        xperm = xs[nodes].T                        # [128, 12500] slot order
        # slot m = 8r+g -> column g*1568+r
        mm = np.arange(NL)
        xt[:, (mm % 8) * R_ROUNDS + mm // 8] = xperm
        xts.append(np.ascontiguousarray(xt))

    # dinv in group layout [128, 1568]: [16g+j, r] = dinv[node 8r+g]; fake 0
    dinv_grps = []
    for c in range(N_CORES):
        dg = np.zeros((128, R_ROUNDS), dtype=np.float32)
        nodes = c * NL + perms[c]
        dvals = np.zeros(NLP, dtype=np.float32)
        dvals[:NL] = dinv[nodes]
        mm = np.arange(NLP)
        # slot m -> (g=m%8, r=m//8); dg[16g+j, r] = dvals[m] for all j
        dmat = np.zeros((8, R_ROUNDS), dtype=np.float32)
        dmat[mm % 8, mm // 8] = dvals
        dg = np.repeat(dmat, 16, axis=0)           # [128, 1568]
        dinv_grps.append(np.ascontiguousarray(dg))

    b1_g = np.tile(b1.astype(np.float32), 8)[:, None]   # [128, 1]
    b2_g = np.tile(b2.astype(np.float32), 8)[:, None]

    in_maps = []
    for c in range(N_CORES):
        in_maps.append({
            "xt": xts[c],
            "idx": idx_maps[c],
            "dinv_g": dinv_grps[c],
            "b1_g": b1_g,
            "b2_g": b2_g,
            "w1": W1.astype(np.float32),
            "w2": W2.astype(np.float32),
        })

    struct = {
        "seg_struct": seg_struct,
        "L_w": L_w,
        "idx_off_w": idx_off_w,
        "colbase": colbase,
        "sc_total": sc_total,
    }
    decode = {
        "cc": cc, "we": we, "g_e": g_e, "i_e": i_e,
        "colbase": colbase, "seg_struct": seg_struct,
    }
    return in_maps, struct, decode


# -------------------------------------------------------------- bass builder
def _build(struct):
    from concourse import bass, mybir, bacc
    import concourse.tile as tile
    from concourse.masks import make_identity

    seg_struct = struct["seg_struct"]
    L_w = struct["L_w"]
    idx_off_w = struct["idx_off_w"]
    colbase = struct["colbase"]
    SC = struct["sc_total"]
    Lsum16 = sum(L_w) // 16

    nc = bacc.Bacc("TRN2", target_bir_lowering=False, debug=False,
                   num_devices=N_CORES)
    f32 = mybir.dt.float32
    xt_in = nc.dram_tensor("xt", [IN_CH, NLP], f32, kind="ExternalInput")
    idx_in = nc.dram_tensor("idx", [128, Lsum16], mybir.dt.int16,
                            kind="ExternalInput")
    dinv_in = nc.dram_tensor("dinv_g", [128, R_ROUNDS], f32,
                             kind="ExternalInput")
    b1_in = nc.dram_tensor("b1_g", [128, 1], f32, kind="ExternalInput")
    b2_in = nc.dram_tensor("b2_g", [128, 1], f32, kind="ExternalInput")
    w1_in = nc.dram_tensor("w1", [IN_CH, HID], f32, kind="ExternalInput")
    w2_in = nc.dram_tensor("w2", [HID, HID], f32, kind="ExternalInput")
    sc_out = nc.dram_tensor("scores", [128, SC], f32, kind="ExternalOutput")

    RG = [list(range(N_CORES))]
    CH = 392  # transform chunk (1568 = 4*392)

    with tile.TileContext(nc) as tc:
        with (
            tc.tile_pool(name="dram", bufs=1, space="DRAM") as dram,
            tc.tile_pool(name="win", bufs=1) as winp,
            tc.tile_pool(name="gb", bufs=2) as gbp,
            tc.tile_pool(name="zt", bufs=1) as ztp,
            tc.tile_pool(name="sm", bufs=1) as smp,
            tc.tile_pool(name="xc", bufs=2) as xcp,
            tc.tile_pool(name="t16", bufs=2) as t16p,
            tc.tile_pool(name="red", bufs=1) as redp,
            tc.tile_pool(name="ps", bufs=2, space="PSUM") as psp,
            tc.tile_pool(name="ps2", bufs=2, space="PSUM") as ps2p,
        ):
            # staged shard tables + allgather outputs (DRAM)
            staged = [dram.tile([HID, NLP], f32, tag=f"stg{i}", name=f"stg{i}") for i in range(3)]
            stagedA = [dram.tile([128, NLP], f32, tag=f"stgA{i}", name=f"stgA{i}") for i in range(3)]

            # small resident tiles
            idx_sb = smp.tile([128, Lsum16], mybir.dt.int16)
            nc.sync.dma_start(out=idx_sb[:, :], in_=idx_in.ap())
            dinv_g = smp.tile([128, R_ROUNDS], f32)
            nc.sync.dma_start(out=dinv_g[:, :], in_=dinv_in.ap())
            bb_g = smp.tile([128, 2], f32)
            nc.sync.dma_start(out=bb_g[:, 0:1], in_=b1_in.ap())
            nc.sync.dma_start(out=bb_g[:, 1:2], in_=b2_in.ap())
            w1_sb = smp.tile([IN_CH, HID], f32)
            nc.sync.dma_start(out=w1_sb[:, :], in_=w1_in.ap())
            w2_sb = smp.tile([HID, HID], f32)
            nc.sync.dma_start(out=w2_sb[:, :], in_=w2_in.ap())
            ident = smp.tile([128, 128], f32)
            make_identity(nc, ident[:, :])

            scores_sb = smp.tile([128, SC], f32)

            def transform(rhs_getter, w_sb, stg, extra_scale=None):
                # computes t[16, cols] = w^T @ rhs per group chunk and
                # writes staged[j, 8r+g] (perm-slot order)
                stg_v = stg[:, :].rearrange("j (r e) -> j r e", e=8)
                for g in range(8):
                    for ci in range(4):
                        c0 = ci * CH
                        rhs = rhs_getter(g, c0)
                        pt = psp.tile([HID, CH], f32, space="PSUM", tag="pt")
                        nc.tensor.matmul(out=pt[:, :], lhsT=w_sb[:, :],
                                         rhs=rhs, start=True, stop=True)
                        tt = t16p.tile([HID, CH], f32, tag="tt")
                        if extra_scale is None:
                            nc.scalar.copy(out=tt[:, :], in_=pt[:, :])
                        else:
                            nc.vector.tensor_tensor(
                                out=tt[:, :], in0=pt[:, :],
                                in1=extra_scale(g, c0),
                                op=mybir.AluOpType.mult)
                        nc.sync.dma_start(out=stg_v[:, c0:c0 + CH, g],
                                          in_=tt[:, :])

            # ---------------- phase 1 table: t1 = x~ @ W1 (dinv pre-folded)
            def rhs_x(g, c0):
                xc = xcp.tile([IN_CH, CH], f32, tag="xc")
                nc.sync.dma_start(
                    out=xc[:, :],
                    in_=xt_in.ap()[:, g * R_ROUNDS + c0:
                                   g * R_ROUNDS + c0 + CH])
                return xc[:, :]

            transform(rhs_x, w1_sb, staged[0])
            nc.gpsimd.collective_compute(
                "AllGather", mybir.AluOpType.bypass, replica_groups=RG,
                ins=[staged[0].opt()], outs=[stagedA[0].opt()])

            def grp_read(stg, dst):
                # dst[16g+j, r] = staged[j, 8r+g]
                sv = stg[:, :].rearrange("j (r e) -> j r e", e=8)
                for g in range(8):
                    nc.sync.dma_start(out=dst[16 * g:16 * (g + 1), :],
                                      in_=sv[:, :, g])

            def sweep(tblA, z, scoring_h=None):
                for w in range(W_WIN):
                    wt = winp.tile([128, WN], f32, tag="wt")
                    av = tblA[:, :].rearrange("(c j) m -> j c m", j=16)
                    for g in range(8):
                        nc.sync.dma_start(
                            out=wt[16 * g:16 * (g + 1), :],
                            in_=av[:, 2 * w:2 * w + 2, :])
                    if scoring_h is None:
                        red = redp.tile([128, R_ROUNDS], f32, tag="red")
                    for si, seg in enumerate(seg_struct[w]):
                        slen = seg["len"]
                        i0 = idx_off_w[w] + seg["base"] // 16
                        gb = gbp.tile([128, SEG], f32, tag="gb")
                        nc.gpsimd.ap_gather(
                            out_ap=gb[:, 0:slen],
                            in_ap=wt[:, :],
                            idxs_ap=idx_sb[:, i0:i0 + slen // 16],
                            channels=128, num_elems=WN, d=1, num_idxs=slen)
                        if scoring_h is None:
                            for (r0, R, Kk, coff) in seg["runs"]:
                                nc.vector.tensor_reduce(
                                    out=red[:, r0:r0 + R],
                                    in_=gb[:, coff:coff + R * Kk].rearrange(
                                        "p (r k) -> p r k", k=Kk),
                                    axis=mybir.AxisListType.X,
                                    op=mybir.AluOpType.add)
                        else:
                            pr = gb
                            for (r0, R, Kk, coff) in seg["runs"]:
                                bb = scoring_h[:, r0:r0 + R].rearrange(
                                    "p (r o) -> p r o", o=1).to_broadcast(
                                    [128, R, Kk])
                                nc.vector.tensor_tensor(
                                    out=pr[:, coff:coff + R * Kk].rearrange(
                                        "p (r k) -> p r k", k=Kk),
                                    in0=gb[:, coff:coff + R * Kk].rearrange(
                                        "p (r k) -> p r k", k=Kk),
                                    in1=bb, op=mybir.AluOpType.mult)
                            ntile = slen // 128
                            cb = colbase[(w, si)]
                            for q0 in range(0, ntile, 4):
                                qn = min(4, ntile - q0)
                                pt2 = ps2p.tile([128, 512], f32,
                                                space="PSUM", tag="pt2")
                                for q in range(qn):
                                    t = q0 + q
                                    nc.tensor.transpose(
                                        out=pt2[:, q * 128:(q + 1) * 128],
                                        in_=pr[:, t * 128:(t + 1) * 128],
                                        identity=ident[:, :])
                                nc.vector.tensor_reduce(
                                    out=scores_sb[:, cb + q0 * 8:
                                                  cb + q0 * 8 + qn * 8],
                                    in_=pt2[:, 0:qn * 128].rearrange(
                                        "p (t g f) -> p t g f", g=8, f=16),
                                    axis=mybir.AxisListType.X,
                                    op=mybir.AluOpType.add)
                    if scoring_h is None:
                        nc.vector.tensor_tensor(
                            out=z[:, :], in0=z[:, :], in1=red[:, :],
                            op=mybir.AluOpType.add)

            # L1 aggregation
            z1 = ztp.tile([128, R_ROUNDS], f32, tag="z1")
            grp_read(staged[0], z1)          # self term
            sweep(stagedA[0], z1)

            # x2~ = relu(z1*dinv + b1) * dinv
            x2g = ztp.tile([128, R_ROUNDS], f32, tag="x2g")
            nc.vector.tensor_tensor(out=x2g[:, :], in0=z1[:, :],
                                    in1=dinv_g[:, :],
                                    op=mybir.AluOpType.mult)
            nc.vector.tensor_tensor(
                out=x2g[:, :], in0=x2g[:, :],
                in1=bb_g[:, 0:1].to_broadcast([128, R_ROUNDS]),
                op=mybir.AluOpType.add)
            nc.scalar.activation(out=x2g[:, :], in_=x2g[:, :],
                                 func=mybir.ActivationFunctionType.Relu)
            nc.vector.tensor_tensor(out=x2g[:, :], in0=x2g[:, :],
                                    in1=dinv_g[:, :],
                                    op=mybir.AluOpType.mult)

            # t2 = x2~ @ W2
            def rhs_x2(g, c0):
                xc2 = xcp.tile([HID, CH], f32, tag="xc2")
                nc.sync.dma_start(out=xc2[:, :],
                                  in_=x2g[16 * g:16 * (g + 1), c0:c0 + CH])
                return xc2[:, :]

            transform(rhs_x2, w2_sb, staged[1])
            nc.gpsimd.collective_compute(
                "AllGather", mybir.AluOpType.bypass, replica_groups=RG,
                ins=[staged[1].opt()], outs=[stagedA[1].opt()])

            z2 = ztp.tile([128, R_ROUNDS], f32, tag="z1")
            grp_read(staged[1], z2)
            sweep(stagedA[1], z2)

            # H = z2*dinv + b2
            hg = ztp.tile([128, R_ROUNDS], f32, tag="x2g")
            nc.vector.tensor_tensor(out=hg[:, :], in0=z2[:, :],
                                    in1=dinv_g[:, :],
                                    op=mybir.AluOpType.mult)
            nc.vector.tensor_tensor(
                out=hg[:, :], in0=hg[:, :],
                in1=bb_g[:, 1:2].to_broadcast([128, R_ROUNDS]),
                op=mybir.AluOpType.add)

            # stagedH[j, 8r+g] = hg[16g+j, r]
            hv = staged[2][:, :].rearrange("j (r e) -> j r e", e=8)
            for g in range(8):
                nc.sync.dma_start(out=hv[:, :, g],
                                  in_=hg[16 * g:16 * (g + 1), :])
            nc.gpsimd.collective_compute(
                "AllGather", mybir.AluOpType.bypass, replica_groups=RG,
                ins=[staged[2].opt()], outs=[stagedA[2].opt()])

            # scoring sweep
            sweep(stagedA[2], None, scoring_h=hg)

            nc.scalar.activation(out=scores_sb[:, :], in_=scores_sb[:, :],
                                 func=mybir.ActivationFunctionType.Sigmoid)
            nc.sync.dma_start(out=sc_out.ap(), in_=scores_sb[:, :])

    nc.compile()
    return nc


# --------------------------------------------------------------- entry point
def kernel(x, edge_index, W1, b1, W2, b2):
    global LAST_HW_EXEC_NS
    x = np.asarray(x, dtype=np.float32)
    edge_index = np.asarray(edge_index)
    W1 = np.asarray(W1, dtype=np.float32)
    b1 = np.asarray(b1, dtype=np.float32)
    W2 = np.asarray(W2, dtype=np.float32)
    b2 = np.asarray(b2, dtype=np.float32)

    in_maps, struct, dec = _prepare(x, edge_index, W1, b1, W2, b2)
    nc = _build(struct)

    # axon_hooks shim for trace (inline so kernel.py is self-contained)
    trace = os.environ.get("BASS_GNN_NOTRACE", "0") != "1"
    if trace:
        try:
            import antenv  # noqa
            if "antenv.axon_hooks" not in sys.modules:
                import importlib.util as ilu
                p = "/opt/trn_rl_repo/antenv/axon_hooks.py"
                if os.path.exists(p):
                    spec = ilu.spec_from_file_location("antenv.axon_hooks", p)
                    m = ilu.module_from_spec(spec)
                    spec.loader.exec_module(m)
                    sys.modules["antenv.axon_hooks"] = m
                else:
                    trace = False
        except Exception:
            trace = False

    from concourse.bass_utils import run_bass_kernel_spmd
    res = run_bass_kernel_spmd(nc, in_maps, core_ids=list(range(N_CORES)),
                               trace=trace)
    LAST_HW_EXEC_NS = res.exec_time_ns

    # decode scores
    cc, we, g_e, i_e = dec["cc"], dec["we"], dec["g_e"], dec["i_e"]
    seg_struct, colbase = dec["seg_struct"], dec["colbase"]
    # map stream pos i -> (seg index, pos within seg) per window
    scores = np.empty(N_EDGES, dtype=np.float32)
    outs = [res.results[c]["scores"] for c in range(N_CORES)]
    for w in range(W_WIN):
        segs = seg_struct[w]
        bases = np.array([s["base"] for s in segs] +
                         [segs[-1]["base"] + segs[-1]["len"]], dtype=np.int64)
        m = we == w
        iw = i_e[m]
        si = np.searchsorted(bases, iw, side="right") - 1
        ip = iw - bases[si]
        tt = ip // 128
        pp = ip % 128
        cbs = np.array([colbase[(w, s)] for s in range(len(segs))],
                       dtype=np.int64)
        col = cbs[si] + tt * 8 + g_e[m]
        vals = np.empty(len(iw), dtype=np.float32)
        ccm = cc[m]
        for c in range(N_CORES):
            mc = ccm == c
            vals[mc] = outs[c][pp[mc], col[mc]]
        scores[m] = vals
    return scores


if __name__ == "__main__":
    import importlib.util as ilu
    spec = ilu.spec_from_file_location("ref", "/root/problem/reference.py")
    ref = ilu.module_from_spec(spec)
    spec.loader.exec_module(ref)
    inputs = {k: np.asarray(v) for k, v in ref.setup_inputs().items()}
    exp = _reference_numpy(inputs["x"].astype(np.float32),
                           inputs["edge_index"], inputs["W1"], inputs["b1"],
                           inputs["W2"], inputs["b2"])
    act = kernel(**inputs)
    rel = np.abs(act - exp) / (np.abs(exp) + 1e-7)
    print(f"Relative error: {rel.max():.3e}")
    print(f"HW exec time: {LAST_HW_EXEC_NS} ns")


# revision 8
# speedup vs baseline: 1.0396x; 1.0396x over previous
"""GCN link-prediction kernel for Trainium2 (8 NeuronCores).

Self-contained. N=100000 nodes, IN_CH=128, HID=16, E=3200000 edges.

Device strategy (transposed-table, ap_gather sweeps):
- Nodes range-sharded over 8 cores; edges partitioned by target owner.
- Tables t = (x~ @ W) stored feature-on-partition, replicated per
  16-partition GPSIMD group; per-edge gathers via InstAPGather over 4
  node windows (int16 index range), padded per (round-of-8-nodes,
  window) rectangles so DVE segment reduces are rectangular.
- dinv folded into gather sources (host folds into x, device folds into
  the layer-2 transform input), so aggregation is an unweighted sum.
- Scoring reuses the same slot streams on the final H table; per-slot
  dot products via DVE mul + PE transpose + DVE reduce; sigmoid on ACT.
- 3 AllGathers (one per table) via gpsimd collective_compute.
"""
import sys
import os

sys.path.insert(0, "/opt/trn_rl_repo")

import numpy as np

N_NODES = 100000
IN_CH = 128
HID = 16
N_EDGES = 3200000
N_CORES = 8
NL = 12500            # real nodes per core
NLP = 12544           # padded (8*1568)
R_ROUNDS = 1568       # rounds of 8 nodes
W_WIN = 4             # table windows
WN = 25088            # nodes per window (2 cores * NLP)
SEG = 3072            # gather slots per group per instruction (mult of 128)

LAST_HW_EXEC_NS = None


# ----------------------------------------------------------------- reference
def _reference_numpy(x, edge_index, W1, b1, W2, b2):
    row = edge_index[0].astype(np.int64)
    col = edge_index[1].astype(np.int64)
    n = x.shape[0]
    deg = np.bincount(col, minlength=n).astype(np.float32) + 1.0
    dinv = 1.0 / np.sqrt(deg)

    def layer(h, W, b):
        hw = h @ W
        g = hw * dinv[:, None]
        agg = np.zeros_like(g)
        np.add.at(agg, col, g[row])
        out = dinv[:, None] * (agg + g) + b
        return out

    h = layer(x, W1, b1)
    h = np.maximum(h, 0.0)
    h = layer(h, W2, b2)
    s = (h[row] * h[col]).sum(axis=1)
    return 1.0 / (1.0 + np.exp(-s))


# ----------------------------------------------------------------- host prep
def _prepare(x, edge_index, W1, b1, W2, b2):
    row = edge_index[0].astype(np.int64)
    col = edge_index[1].astype(np.int64)
    deg = np.bincount(col, minlength=N_NODES).astype(np.float32) + 1.0
    dinv = (1.0 / np.sqrt(deg)).astype(np.float32)

    cc = (col // NL).astype(np.int32)          # owner core of each edge
    rc = (row // NL).astype(np.int32)          # source core
    we = (rc // 2).astype(np.int32)            # window of each edge

    # per-(col, window) counts -> per-core node perm (lexsort by window vec)
    cntkey = col * 4 + we
    cnt = np.bincount(cntkey, minlength=N_NODES * 4).reshape(N_NODES, 4)

    permpos = np.empty(N_NODES, dtype=np.int64)   # global node -> perm slot
    perms = []
    for c in range(N_CORES):
        cn = cnt[c * NL:(c + 1) * NL]
        order = np.lexsort((cn[:, 3], cn[:, 2], cn[:, 1], cn[:, 0]))[::-1]
        perms.append(order)                        # perm slot m -> local node
        permpos[c * NL + order] = np.arange(NL)

    m_col = permpos[col]                           # perm slot of target
    g_e = (m_col % 8).astype(np.int32)
    r_e = (m_col // 8).astype(np.int32)

    # K[r, w] = max over (core, group) of counts, >= 1
    k4key = ((cc.astype(np.int64) * R_ROUNDS + r_e) * 8 + g_e) * 4 + we
    cnt4 = np.bincount(k4key, minlength=N_CORES * R_ROUNDS * 8 * 4)
    cnt4 = cnt4.reshape(N_CORES, R_ROUNDS, 8, 4)
    K = cnt4.max(axis=(0, 2))                      # [R, 4]
    K = np.maximum(K, 1).astype(np.int64)
    # block-max over 4-round blocks: fewer, longer K-runs (fewer DVE ops)
    K = np.repeat(K.reshape(R_ROUNDS // 2, 2, W_WIN).max(axis=1), 2, axis=0)

    # segment structure per window: rounds packed into segments <= SEG,
    # each segment padded to a multiple of 128 slots.
    seg_struct = []      # per w: list of dict(runs=[(r0,R,K,coff)], len, base)
    off_round = np.zeros((R_ROUNDS, W_WIN), dtype=np.int64)  # pos in stream
    L_w = []
    for w in range(W_WIN):
        segs = []
        cur_runs = []
        cur_len = 0
        seg_base = 0
        r = 0
        run_r0, run_k, run_cnt = 0, int(K[0, w]), 0

        def close_run():
            nonlocal run_cnt, cur_runs
            if run_cnt > 0:
                cur_runs.append((run_r0, run_cnt, run_k,
                                 cur_len - run_cnt * run_k))
            run_cnt = 0

        def close_seg():
            nonlocal cur_len, cur_runs, seg_base, segs
            if cur_len == 0:
                return
            plen = -cur_len % 128
            segs.append({"runs": cur_runs, "len": cur_len + plen,
                         "base": seg_base})
            seg_base += cur_len + plen
            cur_runs = []
            cur_len = 0

        while r < R_ROUNDS:
            k = int(K[r, w])
            if cur_len + k > SEG:
                close_run()
                close_seg()
                run_r0, run_k, run_cnt = r, k, 0
            if run_cnt == 0 or k != run_k:
                close_run()
                run_r0, run_k = r, k
            off_round[r, w] = seg_base + cur_len
            cur_len += k
            run_cnt += 1
            r += 1
        close_run()
        close_seg()
        seg_struct.append(segs)
        L_w.append(seg_base)

    # scoring column layout: in sweep order (w, seg): ncols = len/128*8
    colbase = {}
    sc_total = 0
    for w in range(W_WIN):
        for si, s in enumerate(seg_struct[w]):
            colbase[(w, si)] = sc_total
            sc_total += (s["len"] // 128) * 8

    # per-edge slot rank k within (core, w, g, r)
    order = np.lexsort((r_e, g_e, we, cc))
    so = order
    key_sorted = (((cc[so].astype(np.int64) * 4 + we[so]) * 8 + g_e[so])
                  * R_ROUNDS + r_e[so])
    newgrp = np.empty(len(so), dtype=bool)
    newgrp[0] = True
    newgrp[1:] = key_sorted[1:] != key_sorted[:-1]
    starts = np.flatnonzero(newgrp)
    gid = np.cumsum(newgrp) - 1
    k_in = np.arange(len(so)) - starts[gid]
    k_e = np.empty(N_EDGES, dtype=np.int64)
    k_e[so] = k_in

    # device stream position (per group): i = off_round[r, w] + k
    i_e = off_round[r_e, we] + k_e

    # gather index value: window-local gid of source row
    gid_row = rc.astype(np.int64) * NLP + permpos[row]
    widx_e = (gid_row - we.astype(np.int64) * WN).astype(np.int16)

    # build idx arrays per core: [128, sum_w L_w/16] int16, pads -> zero row
    Lsum = sum(L_w)
    idx_off_w = np.cumsum([0] + [lw // 16 for lw in L_w])
    idx_maps = []
    for c in range(N_CORES):
        arr = np.full((128, Lsum // 16), 12500, dtype=np.int16)  # pad idx
        idx_maps.append(arr)
    # scatter edge idx values: core cc, group g, stream pos i, window w:
    # element t of group stream at [16*g + (i%16)?? NO: wrapped (s p):
    # stream elem i at partition 16*g + i%16, col idx_off_w[w] + i//16
    part = (g_e.astype(np.int64) * 16 + (i_e % 16))
    colp = idx_off_w[we] + i_e // 16
    for c in range(N_CORES):
        m = cc == c
        idx_maps[c][part[m], colp[m]] = widx_e[m]

    # x~ = x * dinv, transposed, perm-grouped per core:
    # xt[c][:, g*1568 + r] = x~[node at slot 8r+g of core c]; fakes -> 0
    xs = (x.astype(np.float32) * dinv[:, None])
    xts = []
    for c in range(N_CORES):
        nodes = c * NL + perms[c]                  # perm slot m -> global
        xt = np.zeros((IN_CH, NLP), dtype=np.float32)
        xperm = xs[nodes].T                        # [128, 12500] slot order
        # slot m = 8r+g -> column g*1568+r
        mm = np.arange(NL)
        xt[:, (mm % 8) * R_ROUNDS + mm // 8] = xperm
        xts.append(np.ascontiguousarray(xt))

    # dinv in group layout [128, 1568]: [16g+j, r] = dinv[node 8r+g]; fake 0
    dinv_grps = []
    for c in range(N_CORES):
        dg = np.zeros((128, R_ROUNDS), dtype=np.float32)
        nodes = c * NL + perms[c]
        dvals = np.zeros(NLP, dtype=np.float32)
        dvals[:NL] = dinv[nodes]
        mm = np.arange(NLP)
        # slot m -> (g=m%8, r=m//8); dg[16g+j, r] = dvals[m] for all j
        dmat = np.zeros((8, R_ROUNDS), dtype=np.float32)
        dmat[mm % 8, mm // 8] = dvals
        dg = np.repeat(dmat, 16, axis=0)           # [128, 1568]
        dinv_grps.append(np.ascontiguousarray(dg))

    b1_g = np.tile(b1.astype(np.float32), 8)[:, None]   # [128, 1]
    b2_g = np.tile(b2.astype(np.float32), 8)[:, None]

    in_maps = []
    for c in range(N_CORES):
        in_maps.append({
            "xt": xts[c],
            "idx": idx_maps[c],
            "dinv_g": dinv_grps[c],
            "b1_g": b1_g,
            "b2_g": b2_g,
            "w1": W1.astype(np.float32),
            "w2": W2.astype(np.float32),
        })

    struct = {
        "seg_struct": seg_struct,
        "L_w": L_w,
        "idx_off_w": idx_off_w,
        "colbase": colbase,
        "sc_total": sc_total,
    }
    decode = {
        "cc": cc, "we": we, "g_e": g_e, "i_e": i_e,
        "colbase": colbase, "seg_struct": seg_struct,
    }
    return in_maps, struct, decode


# -------------------------------------------------------------- bass builder
def _build(struct):
    from concourse import bass, mybir, bacc
    import concourse.tile as tile
    from concourse.masks import make_identity

    seg_struct = struct["seg_struct"]
    L_w = struct["L_w"]
    idx_off_w = struct["idx_off_w"]
    colbase = struct["colbase"]
    SC = struct["sc_total"]
    Lsum16 = sum(L_w) // 16

    nc = bacc.Bacc("TRN2", target_bir_lowering=False, debug=False,
                   num_devices=N_CORES)
    f32 = mybir.dt.float32
    xt_in = nc.dram_tensor("xt", [IN_CH, NLP], f32, kind="ExternalInput")
    idx_in = nc.dram_tensor("idx", [128, Lsum16], mybir.dt.int16,
                            kind="ExternalInput")
    dinv_in = nc.dram_tensor("dinv_g", [128, R_ROUNDS], f32,
                             kind="ExternalInput")
    b1_in = nc.dram_tensor("b1_g", [128, 1], f32, kind="ExternalInput")
    b2_in = nc.dram_tensor("b2_g", [128, 1], f32, kind="ExternalInput")
    w1_in = nc.dram_tensor("w1", [IN_CH, HID], f32, kind="ExternalInput")
    w2_in = nc.dram_tensor("w2", [HID, HID], f32, kind="ExternalInput")
    sc_out = nc.dram_tensor("scores", [128, SC], f32, kind="ExternalOutput")

    RG = [list(range(N_CORES))]
    CH = 392  # transform chunk (1568 = 4*392)

    with tile.TileContext(nc) as tc:
        with (
            tc.tile_pool(name="dram", bufs=1, space="DRAM") as dram,
            tc.tile_pool(name="win", bufs=1) as winp,
            tc.tile_pool(name="gb", bufs=2) as gbp,
            tc.tile_pool(name="zt", bufs=1) as ztp,
            tc.tile_pool(name="sm", bufs=1) as smp,
            tc.tile_pool(name="xc", bufs=1) as xcp,
            tc.tile_pool(name="t16", bufs=1) as t16p,
            tc.tile_pool(name="red", bufs=1) as redp,
            tc.tile_pool(name="ps", bufs=2, space="PSUM") as psp,
            tc.tile_pool(name="ps2", bufs=2, space="PSUM") as ps2p,
        ):
            # staged shard tables + allgather outputs (DRAM)
            staged = [dram.tile([HID, NLP], f32, tag=f"stg{i}", name=f"stg{i}") for i in range(3)]
            stagedA = [dram.tile([128, NLP], f32, tag=f"stgA{i}", name=f"stgA{i}") for i in range(3)]

            # small resident tiles
            idx_sb = smp.tile([128, Lsum16], mybir.dt.int16)
            nc.sync.dma_start(out=idx_sb[:, :], in_=idx_in.ap())
            dinv_g = smp.tile([128, R_ROUNDS], f32)
            nc.sync.dma_start(out=dinv_g[:, :], in_=dinv_in.ap())
            bb_g = smp.tile([128, 2], f32)
            nc.sync.dma_start(out=bb_g[:, 0:1], in_=b1_in.ap())
            nc.sync.dma_start(out=bb_g[:, 1:2], in_=b2_in.ap())
            w1_sb = smp.tile([IN_CH, HID], f32)
            nc.sync.dma_start(out=w1_sb[:, :], in_=w1_in.ap())
            w2_sb = smp.tile([HID, HID], f32)
            nc.sync.dma_start(out=w2_sb[:, :], in_=w2_in.ap())
            ident = smp.tile([128, 128], f32)
            make_identity(nc, ident[:, :])

            scores_sb = smp.tile([128, SC], f32)

            def transform(rhs_getter, w_sb, stg, extra_scale=None):
                # computes t[16, cols] = w^T @ rhs per group and writes
                # staged[j, 8r+g] (perm-slot order); one store per group
                stg_v = stg[:, :].rearrange("j (r e) -> j r e", e=8)
                for g in range(8):
                    rhs_full = rhs_getter(g)
                    tt = t16p.tile([HID, R_ROUNDS], f32, tag="tt")
                    for ci in range(4):
                        c0 = ci * CH
                        pt = psp.tile([HID, CH], f32, space="PSUM", tag="pt")
                        nc.tensor.matmul(out=pt[:, :], lhsT=w_sb[:, :],
                                         rhs=rhs_full[:, c0:c0 + CH],
                                         start=True, stop=True)
                        nc.scalar.copy(out=tt[:, c0:c0 + CH], in_=pt[:, :])
                    nc.sync.dma_start(out=stg_v[:, :, g], in_=tt[:, :])

            # ---------------- phase 1 table: t1 = x~ @ W1 (dinv pre-folded)
            def rhs_x(g):
                xc = xcp.tile([IN_CH, R_ROUNDS], f32, tag="xc")
                nc.sync.dma_start(
                    out=xc[:, :],
                    in_=xt_in.ap()[:, g * R_ROUNDS:(g + 1) * R_ROUNDS])
                return xc[:, :]

            transform(rhs_x, w1_sb, staged[0])
            nc.gpsimd.collective_compute(
                "AllGather", mybir.AluOpType.bypass, replica_groups=RG,
                ins=[staged[0].opt()], outs=[stagedA[0].opt()])

            def grp_read(stg, dst):
                # dst[16g+j, r] = staged[j, 8r+g]
                sv = stg[:, :].rearrange("j (r e) -> j r e", e=8)
                for g in range(8):
                    nc.sync.dma_start(out=dst[16 * g:16 * (g + 1), :],
                                      in_=sv[:, :, g])

            def sweep(tblA, z, scoring_h=None):
                for w in range(W_WIN):
                    wt = winp.tile([128, WN], f32, tag="wt")
                    av = tblA[:, :].rearrange("(c j) m -> j c m", j=16)
                    for g in range(8):
                        nc.sync.dma_start(
                            out=wt[16 * g:16 * (g + 1), :],
                            in_=av[:, 2 * w:2 * w + 2, :])
                    if scoring_h is None:
                        red = redp.tile([128, R_ROUNDS], f32, tag="red")
                    for si, seg in enumerate(seg_struct[w]):
                        slen = seg["len"]
                        i0 = idx_off_w[w] + seg["base"] // 16
                        gb = gbp.tile([128, SEG], f32, tag="gb")
                        nc.gpsimd.ap_gather(
                            out_ap=gb[:, 0:slen],
                            in_ap=wt[:, :],
                            idxs_ap=idx_sb[:, i0:i0 + slen // 16],
                            channels=128, num_elems=WN, d=1, num_idxs=slen)
                        if scoring_h is None:
                            for (r0, R, Kk, coff) in seg["runs"]:
                                nc.vector.tensor_reduce(
                                    out=red[:, r0:r0 + R],
                                    in_=gb[:, coff:coff + R * Kk].rearrange(
                                        "p (r k) -> p r k", k=Kk),
                                    axis=mybir.AxisListType.X,
                                    op=mybir.AluOpType.add)
                        else:
                            pr = gb
                            for (r0, R, Kk, coff) in seg["runs"]:
                                bb = scoring_h[:, r0:r0 + R].rearrange(
                                    "p (r o) -> p r o", o=1).to_broadcast(
                                    [128, R, Kk])
                                nc.vector.tensor_tensor(
                                    out=pr[:, coff:coff + R * Kk].rearrange(
                                        "p (r k) -> p r k", k=Kk),
                                    in0=gb[:, coff:coff + R * Kk].rearrange(
                                        "p (r k) -> p r k", k=Kk),
                                    in1=bb, op=mybir.AluOpType.mult)
                            ntile = slen // 128
                            cb = colbase[(w, si)]
                            for q0 in range(0, ntile, 4):
                                qn = min(4, ntile - q0)
                                pt2 = ps2p.tile([128, 512], f32,
                                                space="PSUM", tag="pt2")
                                for q in range(qn):
                                    t = q0 + q
                                    nc.tensor.transpose(
                                        out=pt2[:, q * 128:(q + 1) * 128],
                                        in_=pr[:, t * 128:(t + 1) * 128],
                                        identity=ident[:, :])
                                nc.vector.tensor_reduce(
                                    out=scores_sb[:, cb + q0 * 8:
                                                  cb + q0 * 8 + qn * 8],
                                    in_=pt2[:, 0:qn * 128].rearrange(
                                        "p (t g f) -> p t g f", g=8, f=16),
                                    axis=mybir.AxisListType.X,
                                    op=mybir.AluOpType.add)
                    if scoring_h is None:
                        nc.vector.tensor_tensor(
                            out=z[:, :], in0=z[:, :], in1=red[:, :],
                            op=mybir.AluOpType.add)

            # L1 aggregation
            z1 = ztp.tile([128, R_ROUNDS], f32, tag="z1")
            grp_read(staged[0], z1)          # self term
            sweep(stagedA[0], z1)

            # x2~ = relu(z1*dinv + b1) * dinv
            x2g = ztp.tile([128, R_ROUNDS], f32, tag="x2g")
            nc.vector.tensor_tensor(out=x2g[:, :], in0=z1[:, :],
                                    in1=dinv_g[:, :],
                                    op=mybir.AluOpType.mult)
            nc.vector.tensor_tensor(
                out=x2g[:, :], in0=x2g[:, :],
                in1=bb_g[:, 0:1].to_broadcast([128, R_ROUNDS]),
                op=mybir.AluOpType.add)
            nc.scalar.activation(out=x2g[:, :], in_=x2g[:, :],
                                 func=mybir.ActivationFunctionType.Relu)
            nc.vector.tensor_tensor(out=x2g[:, :], in0=x2g[:, :],
                                    in1=dinv_g[:, :],
                                    op=mybir.AluOpType.mult)

            # t2 = x2~ @ W2
            def rhs_x2(g):
                xc2 = xcp.tile([HID, R_ROUNDS], f32, tag="xc2")
                nc.sync.dma_start(out=xc2[:, :],
                                  in_=x2g[16 * g:16 * (g + 1), :])
                return xc2[:, :]

            transform(rhs_x2, w2_sb, staged[1])
            nc.gpsimd.collective_compute(
                "AllGather", mybir.AluOpType.bypass, replica_groups=RG,
                ins=[staged[1].opt()], outs=[stagedA[1].opt()])

            z2 = ztp.tile([128, R_ROUNDS], f32, tag="z1")
            grp_read(staged[1], z2)
            sweep(stagedA[1], z2)

            # H = z2*dinv + b2
            hg = ztp.tile([128, R_ROUNDS], f32, tag="x2g")
            nc.vector.tensor_tensor(out=hg[:, :], in0=z2[:, :],
                                    in1=dinv_g[:, :],
                                    op=mybir.AluOpType.mult)
            nc.vector.tensor_tensor(
                out=hg[:, :], in0=hg[:, :],
                in1=bb_g[:, 1:2].to_broadcast([128, R_ROUNDS]),
                op=mybir.AluOpType.add)

            # stagedH[j, 8r+g] = hg[16g+j, r]
            hv = staged[2][:, :].rearrange("j (r e) -> j r e", e=8)
            for g in range(8):
                nc.sync.dma_start(out=hv[:, :, g],
                                  in_=hg[16 * g:16 * (g + 1), :])
            nc.gpsimd.collective_compute(
                "AllGather", mybir.AluOpType.bypass, replica_groups=RG,
                ins=[staged[2].opt()], outs=[stagedA[2].opt()])

            # scoring sweep
            sweep(stagedA[2], None, scoring_h=hg)

            nc.scalar.activation(out=scores_sb[:, :], in_=scores_sb[:, :],
                                 func=mybir.ActivationFunctionType.Sigmoid)
            nc.sync.dma_start(out=sc_out.ap(), in_=scores_sb[:, :])

    nc.compile()
    return nc


# --------------------------------------------------------------- entry point
def kernel(x, edge_index, W1, b1, W2, b2):
    global LAST_HW_EXEC_NS
    x = np.asarray(x, dtype=np.float32)
    edge_index = np.asarray(edge_index)
    W1 = np.asarray(W1, dtype=np.float32)
    b1 = np.asarray(b1, dtype=np.float32)
    W2 = np.asarray(W2, dtype=np.float32)
    b2 = np.asarray(b2, dtype=np.float32)

    in_maps, struct, dec = _prepare(x, edge_index, W1, b1, W2, b2)
    nc = _build(struct)

    # axon_hooks shim for trace (inline so kernel.py is self-contained)
    trace = os.environ.get("BASS_GNN_NOTRACE", "0") != "1"
    if trace:
        try:
            import antenv  # noqa
            if "antenv.axon_hooks" not in sys.modules:
                import importlib.util as ilu
                p = "/opt/trn_rl_repo/antenv/axon_hooks.py"
                if os.path.exists(p):
                    spec = ilu.spec_from_file_location("antenv.axon_hooks", p)
                    m = ilu.module_from_spec(spec)
                    spec.loader.exec_module(m)
                    sys.modules["antenv.axon_hooks"] = m
                else:
                    trace = False
        except Exception:
            trace = False

    from concourse.bass_utils import run_bass_kernel_spmd
    res = run_bass_kernel_spmd(nc, in_maps, core_ids=list(range(N_CORES)),
                               trace=trace)
    LAST_HW_EXEC_NS = res.exec_time_ns

    # decode scores
    cc, we, g_e, i_e = dec["cc"], dec["we"], dec["g_e"], dec["i_e"]
    seg_struct, colbase = dec["seg_struct"], dec["colbase"]
    # map stream pos i -> (seg index, pos within seg) per window
    scores = np.empty(N_EDGES, dtype=np.float32)
    outs = [res.results[c]["scores"] for c in range(N_CORES)]
    for w in range(W_WIN):
        segs = seg_struct[w]
        bases = np.array([s["base"] for s in segs] +
                         [segs[-1]["base"] + segs[-1]["len"]], dtype=np.int64)
        m = we == w
        iw = i_e[m]
        si = np.searchsorted(bases, iw, side="right") - 1
        ip = iw - bases[si]
        tt = ip // 128
        pp = ip % 128
        cbs = np.array([colbase[(w, s)] for s in range(len(segs))],
                       dtype=np.int64)
        col = cbs[si] + tt * 8 + g_e[m]
        vals = np.empty(len(iw), dtype=np.float32)
        ccm = cc[m]
        for c in range(N_CORES):
            mc = ccm == c
            vals[mc] = outs[c][pp[mc], col[mc]]
        scores[m] = vals
    return scores


if __name__ == "__main__":
    import importlib.util as ilu
    spec = ilu.spec_from_file_location("ref", "/root/problem/reference.py")
    ref = ilu.module_from_spec(spec)
    spec.loader.exec_module(ref)
    inputs = {k: np.asarray(v) for k, v in ref.setup_inputs().items()}
    exp = _reference_numpy(inputs["x"].astype(np.float32),
                           inputs["edge_index"], inputs["W1"], inputs["b1"],
                           inputs["W2"], inputs["b2"])
    act = kernel(**inputs)
    rel = np.abs(act - exp) / (np.abs(exp) + 1e-7)
    print(f"Relative error: {rel.max():.3e}")
    print(f"HW exec time: {LAST_HW_EXEC_NS} ns")
